# revision 11
# baseline (speedup 1.0000x reference)
"""Causal self-attention (B=2, T=2048, C=1024, H=16) on 8 TRN2 NeuronCores.

Sharding: core = b * 4 + g  ->  batch b, head-group g (4 heads of 64 dims).
Each core computes the qkv projection for its 4 heads, causal attention, and
a partial c_proj contribution; the host sums the 4 partials per batch.

v2: all matmul operands are bf16 (PSUM accumulation stays fp32).  x is cast
to bf16 on the host and loaded pre-transposed straight from DRAM via the
DMA xbar transpose, which removes all PE transposes (35us of PE time that
also confused the HAM clock-gate) and their PSUM->SBUF copies.  The AV
matmuls stream only the causally-live q columns (no pt memset), the
normalization runs one broadcast matmul + one multiply per head pair, and a
burst of dummy matmuls at kernel start warms the PE clock-gate while the
first DMAs are in flight.

Structure: one software-pipelined loop over the four 512-token slices.
Step t emits, interleaved at matmul-group granularity:
  - the tail of slice t-1 (softmax normalization + c_proj + store)
  - attention for q-slice t (causal k-tiles only, both head pairs)
  - x^T DMA loads and V and Q^T,K^T projections for slice t+1

Attention per (pair, q-slice): S^T = K^T q-block (row-packed head pairs,
concurrent in the PE array), one exp per k-tile over both heads via a 3D AP,
GPSIMD affine_select zeroes the causal triangle, AV accumulates O^T[65,512]
whose row 64 is the softmax denominator (ones column in V). Normalization
is deferred off the critical path; O^T then feeds c_proj directly as the
stationary operand - no P or O transposes anywhere.
"""

import sys

sys.path.insert(0, "/opt/trn_rl_repo")

import numpy as np
import ml_dtypes

import concourse.bass as bass
import concourse.mybir as mybir
import concourse.tile as tile
from concourse import bacc
from concourse.bass_utils import run_bass_kernel_spmd

B, T, C = 2, 2048, 1024
H = 16          # total heads
HC = 4          # heads per core
D = 64          # head dim
N_CORES = 8
TT = T // 128   # 16 token tiles
CK = C // 128   # 8 input-feature tiles
QS = T // 512   # 4 q-slices
PAIRS = 2       # head pairs per core

F32 = mybir.dt.float32
F32R = mybir.dt.float32r
BF16 = mybir.dt.bfloat16
EXPF = mybir.ActivationFunctionType.Exp
GE = mybir.AluOpType.is_ge


def build_program():
    nc = bacc.Bacc("TRN2", target_bir_lowering=False, debug=False,
                   num_devices=N_CORES)
    xb = nc.dram_tensor("xb", [T, C], BF16, kind="ExternalInput").ap()
    wqkv = nc.dram_tensor("wqkv", [C, 3 * HC * D], BF16,
                          kind="ExternalInput").ap()
    wp = nc.dram_tensor("wp", [HC * D, C], BF16, kind="ExternalInput").ap()
    ones = nc.dram_tensor("ones", [128, 128], BF16, kind="ExternalInput").ap()
    seld = nc.dram_tensor("sel", [97, 64], F32R, kind="ExternalInput").ap()
    yout = nc.dram_tensor("y", [T, C], F32, kind="ExternalOutput").ap()

    with tile.TileContext(nc) as tc:
        build_kernel(nc, tc, xb, wqkv, wp, ones, seld, yout)
    nc.compile()
    return nc


def head2(ap_2d, o, width):
    """[128, 1024] tile viewed as [128, 2 heads, width] starting at col o."""
    return ap_2d.rearrange("p (h c) -> p h c", h=2)[:, :, o:o + width]


class Weave:
    """Round-robin emitter: interleaves closures from several work lists so
    each engine's in-order stream alternates between independent chains."""

    def __init__(self):
        self.lists = []

    def add(self, ops):
        if ops:
            self.lists.append(list(ops))

    def run(self):
        lists = [l for l in self.lists if l]
        total = sum(len(l) for l in lists)
        emitted = 0
        idx = [0] * len(lists)
        while emitted < total:
            best, bfrac = None, None
            for n, l in enumerate(lists):
                if idx[n] < len(l):
                    frac = idx[n] / len(l)
                    if bfrac is None or frac < bfrac:
                        best, bfrac = n, frac
            lists[best][idx[best]]()
            idx[best] += 1
            emitted += 1
        self.lists = []


def build_kernel(nc, tc, xb, wqkv, wp, ones_d, sel_d, yout):
    from contextlib import ExitStack

    ctx = ExitStack()
    with ctx:
        const = ctx.enter_context(tc.tile_pool(name="const", bufs=1))
        ones = const.tile([128, 128], BF16, tag="ones", name="ones")
        nc.sync.dma_start(ones[:], ones_d[:])
        # [2,128] selector for the per-pair denominator broadcast matmul:
        # row0 -> output partitions 0-63, row1 -> partitions 64-127.
        sel = const.tile([97, 64], F32R, tag="sel", name="sel")
        nc.sync.dma_start(sel[:], sel_d[:])
        wq_sb = []
        for k in range(CK):
            t = const.tile([128, 3 * HC * D], BF16, tag=f"wqkv{k}",
                           name=f"wqkv{k}")
            nc.sync.dma_start(t[:], wqkv[k * 128:(k + 1) * 128, :])
            wq_sb.append(t)
        wp_sb = []
        for p in range(2):
            t = const.tile([128, C], BF16, tag=f"wp{p}", name=f"wp{p}")
            nc.sync.dma_start(t[:], wp[p * 128:(p + 1) * 128, :])
            wp_sb.append(t)

        big = ctx.enter_context(tc.tile_pool(name="big", bufs=1))
        KT = [big.tile([128, T], BF16, tag=f"KT{p}", name=f"KT{p}")
              for p in range(PAIRS)]
        VP = [big.tile([128, HC * (D + 1)], BF16, tag=f"VP{i}",
                       name=f"VP{i}") for i in range(TT)]
        # per-slice rotating tiles (live for ~one pipeline step each)
        xtp = ctx.enter_context(tc.tile_pool(name="xTs", bufs=2))
        qtp = ctx.enter_context(tc.tile_pool(name="QTs", bufs=2))
        otp = ctx.enter_context(tc.tile_pool(name="OTs", bufs=2))
        qt_slice = {}   # ts -> [QT tile per pair]  [128 (2hd x 64d), 512]
        ot_slice = {}   # qs -> [O^T tile per pair] [128 (2hd x 64d), 512]
        # denominator slots: rsg[qs] row 32*(2*pair+hp) holds the AV
        # ones-row (32-aligned: engine partition bases must be 32-aligned,
        # and the broadcast matmul needs lhsT/rhs at the same base); one
        # reciprocal serves the whole q-slice.
        rsg = [big.tile([97, 512], F32, tag=f"rsg{q}", name=f"rsg{q}")
               for q in range(QS)]
        recg = [big.tile([97, 512], F32R, tag=f"recg{q}", name=f"recg{q}")
                for q in range(QS)]

        ptpool = ctx.enter_context(tc.tile_pool(name="pt", bufs=4))
        nrm = ctx.enter_context(tc.tile_pool(name="nrm", bufs=2))
        ypool = ctx.enter_context(tc.tile_pool(name="ysb", bufs=2))
        # PSUM budget (8 banks): s 2x2 + av 2x1 + A-phase/proj/rb 2x1
        sps = ctx.enter_context(tc.tile_pool(name="sps", bufs=2,
                                             space="PSUM"))
        avps = ctx.enter_context(tc.tile_pool(name="avps", bufs=1,
                                              space="PSUM"))
        aps = ctx.enter_context(tc.tile_pool(name="aps", bufs=2,
                                             space="PSUM"))

        # Fill the ones column of every V tile once; v_group only writes
        # the data columns.
        for i in range(TT):
            vp3 = VP[i][:].rearrange("p (h c) -> p h c", c=D + 1)
            nc.gpsimd.memset(vp3[:, :, D:D + 1], 1.0)

        def warmup():
            """Dummy matmuls covering the initial DMA window (~10us), so
            the HAM clock-gate reaches 8/8 before real PE work starts and
            the PE never idles long enough to re-throttle."""
            ps = aps.tile([128, 512], F32, tag="a", name="a")
            n = 128
            for i in range(n):
                nc.tensor.matmul(ps[:, 0:128], ones[:], ones[:],
                                 start=(i == 0), stop=(i == n - 1))

        def emit_proj_ops(ts):
            """A-phase for slice ts: x^T DMA loads, V, Q/K projections.
            Returns a list of closures, each roughly one PE matmul-group."""
            ops = []
            sxT = {}

            def dma_all():
                for k in range(CK):
                    xtk = xtp.tile([128, 512], BF16, tag=f"xT{k}",
                                   name=f"xT{k}")
                    nc.sync.dma_start(
                        xtk[:],
                        xb[ts * 512:(ts + 1) * 512, k * 128:(k + 1) * 128],
                        transpose=True)
                    sxT[k] = xtk

            def v_group(j):
                def f():
                    i = ts * 4 + j
                    ps = aps.tile([128, HC * D], F32, tag="a", name="a")
                    for k in range(CK):
                        nc.tensor.matmul(
                            ps[:],
                            sxT[k][:, j * 128:(j + 1) * 128],
                            wq_sb[k][:, 2 * HC * D:3 * HC * D],
                            start=(k == 0), stop=(k == CK - 1))
                    vp3 = VP[i][:].rearrange("p (h c) -> p h c", c=D + 1)
                    nc.vector.tensor_copy(
                        vp3[:, :, 0:D],
                        ps[:].rearrange("p (h c) -> p h c", c=D))
                return f

            def qk_group(ft):
                def f():
                    ps = aps.tile([128, 512], F32, tag="a", name="a")
                    for k in range(CK):
                        nc.tensor.matmul(
                            ps[:],
                            wq_sb[k][:, ft * 128:(ft + 1) * 128],
                            sxT[k][:],
                            start=(k == 0), stop=(k == CK - 1))
                    if ft < 2:
                        qt = qtp.tile([128, 512], BF16, tag=f"QT{ft}",
                                      name=f"QT{ft}")
                        qt_slice.setdefault(ts, [None, None])[ft] = qt
                        nc.vector.tensor_copy(qt[:], ps[:])
                    else:
                        nc.vector.tensor_copy(
                            KT[ft - 2][:, ts * 512:(ts + 1) * 512], ps[:])
                return f

            ops.append(dma_all)
            for j in range(4):
                ops.append(v_group(j))
            for ft in range(4):
                ops.append(qk_group(ft))
            return ops

        def emit_att_ops(qs):
            """B-phase: attention for q-slice qs, both pairs; AV lagged one
            k-tile behind S so the PE rarely waits on a just-issued exp."""
            ops = []
            nk = 4 * qs + 4
            for pair in range(PAIRS):
                avs = [None, None]
                pts = {}

                def start_pair(pair=pair):
                    for hp in range(2):
                        avs[hp] = avps.tile([65, 512], F32, tag=f"av{hp}",
                                            name=f"av{hp}")
                    if ot_slice.setdefault(qs, [None, None])[pair] is None:
                        ot_slice[qs][pair] = otp.tile(
                            [128, 512], BF16, tag=f"OT{pair}",
                            name=f"OT{pair}")

                def s_exp(ki, pair=pair):
                    def f():
                        o = max(0, 128 * ki - 512 * qs)
                        s = sps.tile([128, 1024], F32, tag="s", name="s")
                        for hp in range(2):
                            nc.tensor.matmul(
                                s[:, hp * 512 + o:hp * 512 + 512],
                                KT[pair][hp * 64:hp * 64 + 64,
                                         ki * 128:(ki + 1) * 128],
                                qt_slice[qs][pair][hp * 64:hp * 64 + 64,
                                                   o:512],
                                start=True, stop=True,
                                tile_position=(hp * 64, 0))
                        pt = ptpool.tile([128, 1024], BF16, tag="pt",
                                         name="pt")
                        nc.scalar.activation(head2(pt[:], o, 512 - o),
                                             head2(s[:], o, 512 - o),
                                             EXPF, scale=0.125)
                        if 128 * ki >= 512 * qs:
                            for hp in range(2):
                                blk = pt[:, hp * 512 + o:hp * 512 + o + 128]
                                nc.gpsimd.affine_select(
                                    out=blk, in_=blk, compare_op=GE,
                                    fill=0.0, base=0, pattern=[[1, 128]],
                                    channel_multiplier=-1)
                        pts[ki] = pt
                    return f

                def av_mm(ki, pair=pair):
                    def f():
                        o = max(0, 128 * ki - 512 * qs)
                        pt = pts.pop(ki)
                        for hp in range(2):
                            h = pair * 2 + hp
                            nc.tensor.matmul(
                                avs[hp][:, o:512],
                                VP[ki][:, h * (D + 1):(h + 1) * (D + 1)],
                                pt[:, hp * 512 + o:hp * 512 + 512],
                                start=(ki == 0), stop=(ki == nk - 1))
                    return f

                def finish_pair(pair=pair):
                    for hp in range(2):
                        nc.vector.tensor_copy(
                            ot_slice[qs][pair][hp * 64:hp * 64 + 64, :],
                            avs[hp][0:64, :])
                        nc.vector.tensor_copy(
                            rsg[qs][32 * (2 * pair + hp):
                                    32 * (2 * pair + hp) + 1, :],
                            avs[hp][64:65, :])

                def op0(pair=pair, start_pair=start_pair, s_exp=s_exp):
                    start_pair()
                    s_exp(0)()

                ops.append(op0)
                for ki in range(1, nk):
                    ops.append(s_exp(ki))
                    ops.append(av_mm(ki - 1))

                def last(pair=pair, av_mm=av_mm, finish_pair=finish_pair,
                         nk=nk):
                    av_mm(nk - 1)()
                    finish_pair()

                ops.append(last)
            return ops

        def emit_tail_ops(qs):
            """Normalize q-slice qs and run its c_proj tiles + store."""
            ops = []

            def recip():
                with nc.allow_low_precision(reason="f32r==f32 bits here"):
                    nc.vector.reciprocal(recg[qs][:], rsg[qs][:])

            ops.append(recip)
            for pair in range(PAIRS):
                def norm(pair=pair):
                    for hp in range(2):
                        r = 32 * (2 * pair + hp)
                        if r < 96:
                            rhs = recg[qs][r:r + 1, :]
                            lhs = sel[r:r + 1, :]
                        else:
                            # matmul operand bases are limited to 0/32/64;
                            # stage the row-96 reciprocal at base 0.
                            rt = nrm.tile([1, 512], F32R, tag="rt",
                                          name="rt")
                            nc.vector.tensor_copy(rt[:], recg[qs][96:97, :])
                            rhs = rt[:]
                            lhs = sel[0:1, :]
                        rb = aps.tile([64, 512], F32, tag="a", name="a")
                        nc.tensor.matmul(rb[:], lhs, rhs,
                                         start=True, stop=True)
                        sl = ot_slice[qs][pair][hp * 64:hp * 64 + 64, :]
                        nc.vector.tensor_mul(sl, sl, rb[:])
                ops.append(norm)
            for i in range(qs * 4, qs * 4 + 4):
                def proj(i=i):
                    yt = ypool.tile([128, C], F32, tag="y", name="y")
                    for cs in range(2):
                        ps = aps.tile([128, 512], F32, tag="a", name="a")
                        for pair in range(PAIRS):
                            nc.tensor.matmul(
                                ps[:],
                                ot_slice[qs][pair][
                                    :, (i - qs * 4) * 128:
                                       (i - qs * 4 + 1) * 128],
                                wp_sb[pair][:, cs * 512:(cs + 1) * 512],
                                start=(pair == 0), stop=(pair == PAIRS - 1))
                        nc.vector.tensor_copy(
                            yt[:, cs * 512:(cs + 1) * 512], ps[:])
                    nc.sync.dma_start(yout[i * 128:(i + 1) * 128, :], yt[:])
                ops.append(proj)
            return ops

        # ---- fused pipeline ----
        warmup()
        for op in emit_proj_ops(0):
            op()
        for t in range(QS):
            w = Weave()
            if t >= 1:
                w.add(emit_tail_ops(t - 1))
            w.add(emit_att_ops(t))
            if t + 1 < QS:
                w.add(emit_proj_ops(t + 1))
            w.run()
        for op in emit_tail_ops(QS - 1):
            op()


_cached_nc = None


def get_program():
    global _cached_nc
    if _cached_nc is None:
        _cached_nc = build_program()
    return _cached_nc


def kernel(x, w_attn, w_proj, _trace=False, _trace_kwargs=None):
    assert x.shape == (B, T, C) and w_attn.shape == (C, 3 * C)
    assert w_proj.shape == (C, C)
    bf16 = ml_dtypes.bfloat16
    x = np.ascontiguousarray(x, dtype=np.float32)
    w_attn = np.ascontiguousarray(w_attn, dtype=np.float32)
    w_proj = np.ascontiguousarray(w_proj, dtype=np.float32)

    sel_host = np.zeros((97, 64), dtype=np.float32)
    sel_host[(0, 32, 64, 96), :] = 1.0
    in_maps = []
    for core in range(N_CORES):
        b, g = divmod(core, 4)
        cols = slice(g * HC * D, (g + 1) * HC * D)
        wqkv = np.concatenate(
            [w_attn[:, 0:C][:, cols], w_attn[:, C:2 * C][:, cols],
             w_attn[:, 2 * C:3 * C][:, cols]], axis=1)
        in_maps.append({
            "xb": np.ascontiguousarray(x[b].astype(bf16)),
            "wqkv": np.ascontiguousarray(wqkv.astype(bf16)),
            "wp": np.ascontiguousarray(w_proj[cols, :].astype(bf16)),
            "ones": np.ones((128, 128), dtype=bf16),
            "sel": sel_host,
        })

    nc = get_program()
    res = run_bass_kernel_spmd(
        nc, in_maps, list(range(N_CORES)),
        trace=_trace, **(_trace_kwargs or {}))

    y = np.zeros((B, T, C), dtype=np.float32)
    for core in range(N_CORES):
        b = core // 4
        y[b] += res.results[core]["y"]
    if _trace:
        return y, res
    return y
